# revision 22
# baseline (speedup 1.0000x reference)
"""Causal self-attention (B=4, T=2048, D=1024, H=16) on 8 TRN2 NeuronCores.

Sharding: data parallel over batch (4 batches x 2 core-pairs) and tensor
parallel over heads (8 heads per core). Each core:
  - projects its batch's tokens to Q/K (feature-major, per-head halves on
    partition halves) and V (token-major via x-stationary matmuls),
  - runs causal attention with per-head row-tiled S matmuls (K=64, both
    heads concurrent on disjoint PE row groups), causal masking via an
    additive -30000 upper-triangular matmul folded into the S psum
    accumulation (exp underflows to 0, no post-exp mask pass),
  - softmax without max-subtraction, denominators from a ones-column in V,
  - pairwise AllGather ships only the partner-needed token half; the out
    projection reads its own half straight from SBUF and accumulates
    own-half products before the collective lands.
Host reassembles the full (4, 2048, 1024) output.
"""

from contextlib import ExitStack

import numpy as np

import concourse.bass as bass
import concourse.mybir as mybir
import concourse.tile as tile
from concourse import bacc, bass_utils
from concourse.bass import ds

N_CORES = 8
B, T, D, H = 4, 2048, 1024, 16
HD = D // H  # 64
FH = 512  # features per core (8 heads)
NFG = 4  # feature groups of 128 (2 heads each) per core
NDS = 8  # 128-row contraction sub-tiles of D
NQC = 4  # 512-query chunks
NTT = 16  # 128-token tiles
TT2 = T // 2
F16 = mybir.dt.float16
BF16 = mybir.dt.bfloat16
F32 = mybir.dt.float32
EXP_SCALE = float(1.0 / np.sqrt(HD))
MASK_NEG = -30000.0


def build_nc(sim_mode=False):
    nc = bacc.Bacc("TRN2", target_bir_lowering=False, debug=False, num_devices=N_CORES)

    xT_d = nc.dram_tensor("xT", (D, T), F16, kind="ExternalInput")
    wq_d = nc.dram_tensor("wq", (D, FH), F16, kind="ExternalInput")
    wk_d = nc.dram_tensor("wk", (D, FH), F16, kind="ExternalInput")
    wv_d = nc.dram_tensor("wv", (D, FH), F16, kind="ExternalInput")
    woa_d = nc.dram_tensor("woa", (FH, D), F16, kind="ExternalInput")
    wob_d = nc.dram_tensor("wob", (FH, D), F16, kind="ExternalInput")
    bq_d = nc.dram_tensor("bq", (NFG, 128, 1), F32, kind="ExternalInput")
    bk_d = nc.dram_tensor("bk", (NFG, 128, 1), F32, kind="ExternalInput")
    bv_d = nc.dram_tensor("bvr", (1, FH), F16, kind="ExternalInput")
    bo_d = nc.dram_tensor("bo", (8, 128, 1), F32, kind="ExternalInput")
    idf_d = nc.dram_tensor("identf", (128, 128), F16, kind="ExternalInput")
    ut_d = nc.dram_tensor("utri", (128, 128), F16, kind="ExternalInput")
    out_d = nc.dram_tensor("out_T", (D, TT2), F32, kind="ExternalOutput")

    with tile.TileContext(nc) as tc:
        with (
            tc.tile_pool(name="const", bufs=1) as cpool,
            tc.tile_pool(name="ofeat", bufs=1) as opool,
            tc.tile_pool(name="psA", bufs=2, space="PSUM") as psA,
            tc.tile_pool(name="dram", bufs=1, space="DRAM") as dram,
        ):
            # small consts first (needed early)
            identf = cpool.tile([128, 128], F16, tag="identf")
            nc.sync.dma_start(identf[:], idf_d[:])
            utri = cpool.tile([128, 128], F16, tag="utri")
            nc.sync.dma_start(utri[:], ut_d[:])
            ones1 = cpool.tile([1, 128], F16, tag="ones1")
            nc.vector.memset(ones1[:], 1.0)
            bqs, bks, bos = [], [], []
            for i in range(NFG):
                bqt = cpool.tile([128, 1], F32, tag=f"bq{i}")
                nc.sync.dma_start(bqt[:], bq_d[i])
                bqs.append(bqt)
                bkt = cpool.tile([128, 1], F32, tag=f"bk{i}")
                nc.sync.dma_start(bkt[:], bk_d[i])
                bks.append(bkt)
            bvr = cpool.tile([1, FH], F16, tag="bvr")
            nc.sync.dma_start(bvr[:], bv_d[:])
            for i in range(8):
                bot = cpool.tile([128, 1], F32, tag=f"bo{i}")
                nc.sync.dma_start(bot[:], bo_d[i])
                bos.append(bot)

            # O_feat: per-fg [128 feat, 2048 tok] fp16, feature-major
            o_feat = []
            for fg in range(NFG):
                of = opool.tile([128, T], F16, tag=f"ofeat{fg}", name=f"ofeat{fg}")
                o_feat.append(of)

            with (
                tc.tile_pool(name="wqkv", bufs=1) as wpool,
                tc.tile_pool(name="xt", bufs=1) as xpool,
                tc.tile_pool(name="qk", bufs=2) as qkpool,
                tc.tile_pool(name="vst", bufs=1) as vpool,
                tc.tile_pool(name="pp", bufs=26) as ppool,
                tc.tile_pool(name="misc", bufs=8) as mpool,
                tc.tile_pool(name="wo", bufs=1) as wopool,
                tc.tile_pool(name="att", bufs=1) as apool,
                tc.tile_pool(name="outs", bufs=4) as outpool,
            ):
                # resident xT (column-chunked, interleaved with wv so the
                # first V/QK chains start after ~1/4 of the x traffic)
                xts, wts = [], {}
                for dsub in range(NDS):
                    xt = xpool.tile([128, T], F16, tag=f"xt{dsub}")
                    nc.sync.dma_start(
                        xt[:, 0:512], xT_d[128 * dsub : 128 * (dsub + 1), 0:512]
                    )
                    wt = wpool.tile([128, FH], F16, tag=f"wv{dsub}")
                    nc.gpsimd.dma_start(wt[:], wv_d[128 * dsub : 128 * (dsub + 1), :])
                    xts.append(xt)
                    wts[("v", dsub)] = wt
                for chunk in range(1, 4):
                    t0c = 512 * chunk
                    for dsub in range(NDS):
                        nc.sync.dma_start(
                            xts[dsub][:, t0c : t0c + 512],
                            xT_d[128 * dsub : 128 * (dsub + 1), t0c : t0c + 512],
                        )
                for pname, wd in (("q", wq_d), ("k", wk_d)):
                    for dsub in range(NDS):
                        wt = wpool.tile([128, FH], F16, tag=f"w{pname}{dsub}")
                        nc.gpsimd.dma_start(wt[:], wd[128 * dsub : 128 * (dsub + 1), :])
                        wts[(pname, dsub)] = wt
                # resident Wo (own rows + partner rows), prefetched up front
                woa, wob = [], []
                for fs in range(4):
                    wt = wopool.tile([128, D], F16, tag=f"woa{fs}")
                    nc.sync.dma_start(wt[:], woa_d[128 * fs : 128 * (fs + 1), :])
                    woa.append(wt)
                for fs in range(4):
                    wt = wopool.tile([128, D], F16, tag=f"wob{fs}")
                    nc.sync.dma_start(wt[:], wob_d[128 * fs : 128 * (fs + 1), :])
                    wob.append(wt)

                # V store: per t-tile [128 tok, 520]: 8x(64 v | 1.0)
                vstore = []
                for tt in range(NTT):
                    vt = vpool.tile([128, 8 * 65], F16, tag=f"vst{tt}")
                    vstore.append(vt)

                def v_chain(tt):
                    ps = psA.tile([128, FH], F32, tag="proj")
                    for dsub in range(NDS):
                        nc.tensor.matmul(
                            ps[:],
                            xts[dsub][:, 128 * tt : 128 * (tt + 1)],
                            wts[("v", dsub)][:],
                            start=(dsub == 0),
                            stop=False,
                        )
                    # bias row broadcast along tokens via K=1 ones matmul
                    nc.tensor.matmul(ps[:], ones1[:], bvr[:], start=False, stop=True)
                    nc.gpsimd.memset(vstore[tt][:], 1.0)
                    nc.vector.tensor_copy(
                        vstore[tt][:].rearrange("p (g c) -> p g c", g=8)[:, :, 0:64],
                        ps[:].rearrange("p (g c) -> p g c", g=8),
                    )

                def qk_chain(pname, fg, tch, dst, bias):
                    f0 = 128 * fg
                    t0 = 512 * tch
                    ps = psA.tile([128, 512], F32, tag="proj")
                    for dsub in range(NDS):
                        nc.tensor.matmul(
                            ps[:],
                            wts[(pname, dsub)][:, f0 : f0 + 128],
                            xts[dsub][:, t0 : t0 + 512],
                            start=(dsub == 0),
                            stop=(dsub == NDS - 1),
                        )
                    nc.vector.tensor_scalar_add(dst[:, t0 : t0 + 512], ps[:], bias[:])

                # filler worklist: emitters for PE work to slot into the
                # ACT-bound attention phase
                filler = []

                def fill(k=1):
                    for _ in range(k):
                        if filler:
                            filler.pop(0)()

                # upfront: V t-tiles 0..7, fg0 Q/K
                qkd = {}  # (fg, 'q'/'k') -> tile
                qkd[(0, "q")] = qkpool.tile([128, T], F16, tag="qd", name="qd0")
                qkd[(0, "k")] = qkpool.tile([128, T], F16, tag="kd", name="kd0")
                for chunk in range(4):
                    for tt in range(4 * chunk, 4 * chunk + 2):
                        v_chain(tt)
                    for pname in ("q", "k"):
                        qk_chain(pname, 0, chunk, qkd[(0, pname)],
                                 (bqs if pname == "q" else bks)[0])
                    for tt in range(4 * chunk + 2, 4 * chunk + 4):
                        v_chain(tt)

                _ps_stack = ExitStack()
                psS = _ps_stack.enter_context(
                    tc.tile_pool(name="psS", bufs=2, space="PSUM")
                )
                psO = _ps_stack.enter_context(
                    tc.tile_pool(name="psO", bufs=2, space="PSUM")
                )

                cc_bufs = []  # (cc_out_flat, sem info) per half
                pid = None if sim_mode else nc.gpsimd.partition_id()
                # token offset of this core's half (0 or 1024)
                if sim_mode:
                    poff, opp, roff = 0, TT2, 256
                else:
                    poff = (pid % 2) * TT2
                    opp = ((pid + 1) % 2) * TT2
                    roff = ((pid + 1) % 2) * 256

                att_own = []

                for fg in range(NFG):
                    # allocate next fg's q/k tiles and queue its proj chains
                    if fg + 1 < NFG:
                        qkd[(fg + 1, "q")] = qkpool.tile([128, T], F16, tag="qd", name=f"qd{fg+1}")
                        qkd[(fg + 1, "k")] = qkpool.tile([128, T], F16, tag="kd", name=f"kd{fg+1}")
                        for pname in ("q", "k"):
                            for tch in range(4):
                                filler.append(
                                    lambda p=pname, f=fg + 1, t=tch: qk_chain(
                                        p, f, t, qkd[(f, p)],
                                        (bqs if p == "q" else bks)[f])
                                )
                    qd, kd = qkd[(fg, "q")], qkd[(fg, "k")]

                    def s_group(j, grp, qd, kd):
                        """Emit one [128k x 1024q] S psum group per head."""
                        q0 = 512 * j
                        pss = {}
                        for hl in range(2):
                            pss[hl] = psS.tile([128, 1024], F32, tag="s", name=f"s{hl}")
                        for ki in range(2):
                            kb = 2 * grp + ki
                            diag = kb >= 4 * j  # triangular tile on diagonal
                            if diag:
                                c0 = 512 * ki + 128 * (kb - 4 * j)
                                for hl in range(2):
                                    nc.tensor.matmul(
                                        pss[hl][:, c0 : c0 + 128],
                                        identf[:],
                                        utri[:],
                                        start=True,
                                        stop=False,
                                    )
                            for hl in range(2):
                                h0 = 64 * hl
                                nc.tensor.matmul(
                                    pss[hl][:, 512 * ki : 512 * (ki + 1)],
                                    kd[h0 : h0 + 64, 128 * kb : 128 * (kb + 1)],
                                    qd[h0 : h0 + 64, q0 : q0 + 512],
                                    start=not diag,
                                    stop=True,
                                )
                        return pss

                    def sx_group(j, grp, qd=qd, kd=kd):
                        """S matmuls + exp for one group; returns hl->P tile."""
                        pss = s_group(j, grp, qd, kd)
                        out = {}
                        for hl in range(2):
                            pt = ppool.tile([128, 1024], F16, tag="p")
                            nc.scalar.activation(
                                pt[:],
                                pss[hl][:],
                                mybir.ActivationFunctionType.Exp,
                                scale=EXP_SCALE,
                            )
                            out[hl] = pt
                        return out

                    # software pipeline over query chunks: S/exp of chunk
                    # j+1 is emitted between the PV blocks of chunk j so the
                    # PE stream stays dense (keeps the HAM clock warm)
                    p_tiles = {}
                    for grp in range(2):
                        for hl, pt in sx_group(0, grp).items():
                            p_tiles[(hl, grp)] = pt
                    deferred = []  # last group of the current j
                    for j in range(NQC):
                        nxt = list(range(2 * (j + 2))) if j + 1 < NQC else []
                        prefetch = nxt[: len(nxt) - 1]
                        p_next = {}
                        for i in range(4):
                            if i in (1, 2) and deferred:
                                grp = deferred.pop(0)
                                for hl, pt in sx_group(j, grp).items():
                                    p_tiles[(hl, grp)] = pt
                            qt = 4 * j + i
                            pso = psO.tile([128, 130], F32, tag="o")
                            nkb = 4 * j + i
                            for hl in range(2):
                                for kb in range(nkb + 1):
                                    grp, ki = kb // 2, kb % 2
                                    c0 = 512 * ki + 128 * i
                                    nc.tensor.matmul(
                                        pso[:, 65 * hl : 65 * hl + 65],
                                        p_tiles[(hl, grp)][:, c0 : c0 + 128],
                                        vstore[kb][:, 130 * fg + 65 * hl : 130 * fg + 65 * hl + 65],
                                        start=(kb == 0),
                                        stop=(kb == nkb),
                                    )
                            psv = pso[:].rearrange("p (h c) -> p h c", h=2)
                            rec = mpool.tile([128, 2], F32, tag="rec")
                            nc.vector.reciprocal(rec[:], psv[:, :, 64])
                            ot = mpool.tile([128, 128], F16, tag="otok")
                            rec_b = bass.AP(
                                rec[:].tensor, rec[:].offset,
                                [rec[:].ap[0], [1, 2], [0, 64]],
                            )
                            nc.vector.tensor_tensor(
                                ot[:].rearrange("p (h c) -> p h c", h=2),
                                psv[:, :, 0:64],
                                rec_b,
                                mybir.AluOpType.mult,
                            )
                            pst = psO.tile([128, 128], F16, tag="o")
                            nc.tensor.transpose(pst[:], ot[:], identf[:])
                            nc.vector.tensor_copy(
                                o_feat[fg][:, 128 * qt : 128 * (qt + 1)], pst[:]
                            )
                            # prefetch next chunk's S/exp groups to keep the
                            # PE stream dense
                            take = (len(prefetch) + (3 - i)) // (4 - i)
                            for _ in range(take):
                                grp = prefetch.pop(0)
                                for hl, pt in sx_group(j + 1, grp).items():
                                    p_next[(hl, grp)] = pt
                            fill(1)
                        p_tiles = p_next
                        deferred = [2 * (j + 1) + 1] if j + 1 < NQC else []

                    # own-half staging for the out projection
                    at = apool.tile([128, TT2], F16, tag=f"attown{fg}")
                    if sim_mode:
                        nc.sync.dma_start(at[:], o_feat[fg][:, 0:TT2])
                    else:
                        nc.gpsimd.dma_start(at[:], o_feat[fg][:, ds(poff, TT2)])
                    att_own.append(at)

                    # pairwise exchange of the partner-needed halves, split
                    # into three collectives (fg 0-1 mid-kernel, fg2, fg3)
                    # so only the last small one can expose latency
                    if fg in (1, 2, 3):
                        fgis = (0, 1) if fg == 1 else (fg,)
                        nr = 128 * len(fgis)
                        cc_in = dram.tile([nr, TT2], F16)
                        cc_out = dram.tile([2, nr, TT2], F16)
                        for k, fgi in enumerate(fgis):
                            if sim_mode:
                                nc.sync.dma_start(
                                    cc_in[128 * k : 128 * (k + 1), :],
                                    o_feat[fgi][:, TT2 : T],
                                )
                            else:
                                nc.gpsimd.dma_start(
                                    cc_in[128 * k : 128 * (k + 1), :],
                                    o_feat[fgi][:, ds(opp, TT2)],
                                )
                        if sim_mode:
                            nc.sync.dma_start(cc_out[0], cc_in[:])
                            nc.sync.dma_start(cc_out[1], cc_in[:])
                        else:
                            nc.gpsimd.collective_compute(
                                "AllGather",
                                mybir.AluOpType.bypass,
                                replica_groups=[[0, 1], [2, 3], [4, 5], [6, 7]],
                                ins=[cc_in.opt()],
                                outs=[cc_out.opt()],
                            )
                        cc_bufs.append((cc_out, nr))

                # drain any leftover filler work
                fill(len(filler))
                _ps_stack.close()  # free attention psum banks for out-proj

                # partner-half attention tiles from the exchanged buffers
                att_p = []
                for bi, (cc_out, nr) in enumerate(cc_bufs):
                    cc_flat = cc_out[:].rearrange("s p t -> (s p) t")
                    pbase = nr if sim_mode else ((pid + 1) % 2) * nr
                    for k in range(nr // 128):
                        at = apool.tile([128, TT2], F16, tag=f"attp{len(att_p)}")
                        if sim_mode:
                            nc.gpsimd.dma_start(
                                at[:], cc_flat[nr + 128 * k : nr + 128 * (k + 1), :]
                            )
                        else:
                            nc.gpsimd.dma_start(
                                at[:], cc_flat[ds(pbase + 128 * k, 128), :]
                            )
                        att_p.append(at)

                # out projection: own-half products first, partner after.
                # Wide psum ring so several chains can accumulate their
                # own-half products while the last exchange is in flight.
                psB = _ps_stack.enter_context(
                    tc.tile_pool(name="psB", bufs=5, space="PSUM")
                )
                for dt_ in range(8):
                    for tch in range(2):
                        t0 = 512 * tch
                        ps = psB.tile([128, 512], F32, tag="oproj")
                        for fs in range(4):
                            nc.tensor.matmul(
                                ps[:],
                                woa[fs][:, 128 * dt_ : 128 * (dt_ + 1)],
                                att_own[fs][:, t0 : t0 + 512],
                                start=(fs == 0),
                                stop=False,
                            )
                        for fs in range(4):
                            nc.tensor.matmul(
                                ps[:],
                                wob[fs][:, 128 * dt_ : 128 * (dt_ + 1)],
                                att_p[fs][:, t0 : t0 + 512],
                                start=False,
                                stop=(fs == 3),
                            )
                        ob = outpool.tile([128, 512], F32, tag="ob")
                        nc.vector.tensor_scalar_add(ob[:], ps[:], bos[dt_][:])
                        nc.sync.dma_start(
                            out_d[128 * dt_ : 128 * (dt_ + 1), t0 : t0 + 512], ob[:]
                        )
                _ps_stack.close()

    nc.compile()
    return nc


def _prep_inputs(x, Wq, bq, Wk, bk, Wv, bv, Wo, bo):
    """Build the 8 per-core input maps."""
    x = np.asarray(x)
    ident = np.eye(128, dtype=np.float32)
    r = np.arange(128)
    utri = np.where(r[None, :] < r[:, None], np.float32(MASK_NEG), np.float32(0.0))
    bo_r = np.asarray(bo).astype(np.float32).reshape(8, 128, 1)
    Wo = np.asarray(Wo).astype(np.float16)

    in_maps = []
    for c in range(N_CORES):
        b = c // 2
        hs = (c % 2) * FH
        ps = FH - hs  # partner's feature offset
        in_maps.append(
            {
                "xT": np.ascontiguousarray(x[b].T).astype(np.float16),
                "wq": np.asarray(Wq)[:, hs : hs + FH].astype(np.float16),
                "wk": np.asarray(Wk)[:, hs : hs + FH].astype(np.float16),
                "wv": np.asarray(Wv)[:, hs : hs + FH].astype(np.float16),
                "woa": np.ascontiguousarray(Wo[hs : hs + FH, :]),
                "wob": np.ascontiguousarray(Wo[ps : ps + FH, :]),
                "bq": np.asarray(bq)[hs : hs + FH].astype(np.float32).reshape(4, 128, 1),
                "bk": np.asarray(bk)[hs : hs + FH].astype(np.float32).reshape(4, 128, 1),
                "bvr": np.asarray(bv)[hs : hs + FH].astype(np.float16).reshape(1, FH),
                "bo": bo_r,
                "identf": ident.astype(np.float16),
                "utri": utri.astype(np.float16),
            }
        )
    return in_maps


_NC_CACHE = None


def kernel(x, Wq, bq, Wk, bk, Wv, bv, Wo, bo):
    global _NC_CACHE
    if _NC_CACHE is None:
        _NC_CACHE = build_nc()
    nc = _NC_CACHE
    in_maps = _prep_inputs(x, Wq, bq, Wk, bk, Wv, bv, Wo, bo)
    res = bass_utils.run_bass_kernel_spmd(nc, in_maps, core_ids=list(range(N_CORES)))
    out = np.empty((B, T, D), dtype=np.float32)
    for c in range(N_CORES):
        b = c // 2
        half = c % 2
        out[b, half * TT2 : (half + 1) * TT2, :] = res.results[c]["out_T"].T
    return out


# revision 26
# speedup vs baseline: 1.3400x; 1.3400x over previous
"""Causal self-attention (B=4, T=2048, D=1024, H=16) on 8 TRN2 NeuronCores.

Sharding: data parallel over batch (4 batches x 2 core-pairs) and tensor
parallel over heads (8 heads per core). Each core:
  - projects its batch's tokens to Q/K (feature-major, per-head halves on
    partition halves) and V (token-major via x-stationary matmuls),
  - runs causal attention with per-head row-tiled S matmuls (K=64, both
    heads concurrent on disjoint PE row groups), causal masking via an
    additive -30000 upper-triangular matmul folded into the S psum
    accumulation (exp underflows to 0, no post-exp mask pass),
  - softmax without max-subtraction, denominators from a ones-column in V,
  - pairwise AllGather ships only the partner-needed token half; the out
    projection reads its own half straight from SBUF and accumulates
    own-half products before the collective lands.
Host reassembles the full (4, 2048, 1024) output.
"""

from contextlib import ExitStack

import numpy as np

import concourse.bass as bass
import concourse.mybir as mybir
import concourse.tile as tile
from concourse import bacc, bass_utils
from concourse.bass import ds

N_CORES = 8
B, T, D, H = 4, 2048, 1024, 16
HD = D // H  # 64
FH = 512  # features per core (8 heads)
NFG = 4  # feature groups of 128 (2 heads each) per core
NDS = 8  # 128-row contraction sub-tiles of D
NQC = 4  # 512-query chunks
NTT = 16  # 128-token tiles
TT2 = T // 2
F16 = mybir.dt.float16
BF16 = mybir.dt.bfloat16
F32 = mybir.dt.float32
EXP_SCALE = float(1.0 / np.sqrt(HD))
MASK_NEG = -30000.0


def build_nc(sim_mode=False):
    nc = bacc.Bacc("TRN2", target_bir_lowering=False, debug=False, num_devices=N_CORES)

    xT_d = nc.dram_tensor("xT", (D, T), F16, kind="ExternalInput")
    wq_d = nc.dram_tensor("wq", (D, FH), F16, kind="ExternalInput")
    wk_d = nc.dram_tensor("wk", (D, FH), F16, kind="ExternalInput")
    wv_d = nc.dram_tensor("wv", (D, FH), F16, kind="ExternalInput")
    woa_d = nc.dram_tensor("woa", (FH, D), F16, kind="ExternalInput")
    wob_d = nc.dram_tensor("wob", (FH, D), F16, kind="ExternalInput")
    bq_d = nc.dram_tensor("bq", (NFG, 128, 1), F32, kind="ExternalInput")
    bk_d = nc.dram_tensor("bk", (NFG, 128, 1), F32, kind="ExternalInput")
    bv_d = nc.dram_tensor("bvr", (1, FH), F16, kind="ExternalInput")
    bo_d = nc.dram_tensor("bo", (8, 128, 1), F32, kind="ExternalInput")
    idf_d = nc.dram_tensor("identf", (128, 128), F16, kind="ExternalInput")
    ut_d = nc.dram_tensor("utri", (128, 128), F16, kind="ExternalInput")
    out_d = nc.dram_tensor("out_T", (D, TT2), F32, kind="ExternalOutput")

    with tile.TileContext(nc) as tc:
        with (
            tc.tile_pool(name="const", bufs=1) as cpool,
            tc.tile_pool(name="ofeat", bufs=1) as opool,
            tc.tile_pool(name="psA", bufs=2, space="PSUM") as psA,
            tc.tile_pool(name="dram", bufs=1, space="DRAM") as dram,
        ):
            # small consts first (needed early)
            identf = cpool.tile([128, 128], F16, tag="identf")
            nc.sync.dma_start(identf[:], idf_d[:])
            utri = cpool.tile([128, 128], F16, tag="utri")
            nc.sync.dma_start(utri[:], ut_d[:])
            ones1 = cpool.tile([1, 128], F16, tag="ones1")
            nc.vector.memset(ones1[:], 1.0)
            bqs, bks, bos = [], [], []
            for i in range(NFG):
                bqt = cpool.tile([128, 1], F32, tag=f"bq{i}")
                nc.sync.dma_start(bqt[:], bq_d[i])
                bqs.append(bqt)
                bkt = cpool.tile([128, 1], F32, tag=f"bk{i}")
                nc.sync.dma_start(bkt[:], bk_d[i])
                bks.append(bkt)
            bvr = cpool.tile([1, FH], F16, tag="bvr")
            nc.sync.dma_start(bvr[:], bv_d[:])
            for i in range(8):
                bot = cpool.tile([128, 1], F32, tag=f"bo{i}")
                nc.sync.dma_start(bot[:], bo_d[i])
                bos.append(bot)

            # O_feat: per-fg [128 feat, 2048 tok] fp16, feature-major
            o_feat = []
            for fg in range(NFG):
                of = opool.tile([128, T], F16, tag=f"ofeat{fg}", name=f"ofeat{fg}")
                o_feat.append(of)

            with (
                tc.tile_pool(name="wqkv", bufs=1) as wpool,
                tc.tile_pool(name="xt", bufs=1) as xpool,
                tc.tile_pool(name="qk", bufs=2) as qkpool,
                tc.tile_pool(name="vst", bufs=1) as vpool,
                tc.tile_pool(name="pp", bufs=26) as ppool,
                tc.tile_pool(name="misc", bufs=8) as mpool,
                tc.tile_pool(name="wo", bufs=1) as wopool,
                tc.tile_pool(name="att", bufs=1) as apool,
                tc.tile_pool(name="outs", bufs=4) as outpool,
            ):
                # resident xT (column-chunked, interleaved with wv so the
                # first V/QK chains start after ~1/4 of the x traffic)
                xts, wts = [], {}
                for dsub in range(NDS):
                    xt = xpool.tile([128, T], F16, tag=f"xt{dsub}")
                    nc.sync.dma_start(
                        xt[:, 0:512], xT_d[128 * dsub : 128 * (dsub + 1), 0:512]
                    )
                    wt = wpool.tile([128, FH], F16, tag=f"wv{dsub}")
                    nc.gpsimd.dma_start(wt[:], wv_d[128 * dsub : 128 * (dsub + 1), :])
                    xts.append(xt)
                    wts[("v", dsub)] = wt
                for chunk in range(1, 4):
                    t0c = 512 * chunk
                    for dsub in range(NDS):
                        nc.sync.dma_start(
                            xts[dsub][:, t0c : t0c + 512],
                            xT_d[128 * dsub : 128 * (dsub + 1), t0c : t0c + 512],
                        )
                for pname, wd in (("q", wq_d), ("k", wk_d)):
                    for dsub in range(NDS):
                        wt = wpool.tile([128, FH], F16, tag=f"w{pname}{dsub}")
                        nc.gpsimd.dma_start(wt[:], wd[128 * dsub : 128 * (dsub + 1), :])
                        wts[(pname, dsub)] = wt
                # resident Wo (own rows + partner rows), prefetched up front
                woa, wob = [], []
                for fs in range(4):
                    wt = wopool.tile([128, D], F16, tag=f"woa{fs}")
                    nc.sync.dma_start(wt[:], woa_d[128 * fs : 128 * (fs + 1), :])
                    woa.append(wt)
                for fs in range(4):
                    wt = wopool.tile([128, D], F16, tag=f"wob{fs}")
                    nc.sync.dma_start(wt[:], wob_d[128 * fs : 128 * (fs + 1), :])
                    wob.append(wt)

                # V store: per t-tile [128 tok, 520]: 8x(64 v | 1.0)
                vstore = []
                for tt in range(NTT):
                    vt = vpool.tile([128, 8 * 65], F16, tag=f"vst{tt}")
                    vstore.append(vt)

                def v_chain(tt):
                    ps = psA.tile([128, FH], F32, tag="proj")
                    for dsub in range(NDS):
                        nc.tensor.matmul(
                            ps[:],
                            xts[dsub][:, 128 * tt : 128 * (tt + 1)],
                            wts[("v", dsub)][:],
                            start=(dsub == 0),
                            stop=False,
                        )
                    # bias row broadcast along tokens via K=1 ones matmul
                    nc.tensor.matmul(ps[:], ones1[:], bvr[:], start=False, stop=True)
                    nc.gpsimd.memset(vstore[tt][:], 1.0)
                    nc.vector.tensor_copy(
                        vstore[tt][:].rearrange("p (g c) -> p g c", g=8)[:, :, 0:64],
                        ps[:].rearrange("p (g c) -> p g c", g=8),
                    )

                def qk_chain(pname, fg, tch, dst, bias):
                    f0 = 128 * fg
                    t0 = 512 * tch
                    ps = psA.tile([128, 512], F32, tag="proj")
                    for dsub in range(NDS):
                        nc.tensor.matmul(
                            ps[:],
                            wts[(pname, dsub)][:, f0 : f0 + 128],
                            xts[dsub][:, t0 : t0 + 512],
                            start=(dsub == 0),
                            stop=(dsub == NDS - 1),
                        )
                    nc.vector.tensor_scalar_add(dst[:, t0 : t0 + 512], ps[:], bias[:])

                # filler worklist: emitters for PE work to slot into the
                # ACT-bound attention phase
                filler = []

                def fill(k=1):
                    for _ in range(k):
                        if filler:
                            filler.pop(0)()

                # upfront: V t-tiles 0..7, fg0 Q/K
                qkd = {}  # (fg, 'q'/'k') -> tile
                qkd[(0, "q")] = qkpool.tile([128, T], F16, tag="qd", name="qd0")
                qkd[(0, "k")] = qkpool.tile([128, T], F16, tag="kd", name="kd0")
                for chunk in range(4):
                    for tt in range(4 * chunk, 4 * chunk + 2):
                        v_chain(tt)
                    for pname in ("q", "k"):
                        qk_chain(pname, 0, chunk, qkd[(0, pname)],
                                 (bqs if pname == "q" else bks)[0])
                    for tt in range(4 * chunk + 2, 4 * chunk + 4):
                        v_chain(tt)

                _ps_stack = ExitStack()
                psS = _ps_stack.enter_context(
                    tc.tile_pool(name="psS", bufs=2, space="PSUM")
                )
                psO = _ps_stack.enter_context(
                    tc.tile_pool(name="psO", bufs=2, space="PSUM")
                )

                cc_bufs = []  # (cc_out_flat, sem info) per half
                pid = None if sim_mode else nc.gpsimd.partition_id()
                # token offset of this core's half (0 or 1024)
                if sim_mode:
                    poff, opp, roff = 0, TT2, 256
                else:
                    poff = (pid % 2) * TT2
                    opp = ((pid + 1) % 2) * TT2
                    roff = ((pid + 1) % 2) * 256

                att_own = []

                for fg in range(NFG):
                    if fg + 1 < NFG:
                        qkd[(fg + 1, "q")] = qkpool.tile([128, T], F16, tag="qd", name=f"qd{fg+1}")
                        qkd[(fg + 1, "k")] = qkpool.tile([128, T], F16, tag="kd", name=f"kd{fg+1}")
                    qd, kd = qkd[(fg, "q")], qkd[(fg, "k")]

                    def s_ki(j, grp, ki, pss, qd=qd, kd=kd):
                        """One key-half of an S group (dense N=512 matmuls)."""
                        q0 = 512 * j
                        kb = 2 * grp + ki
                        diag = kb >= 4 * j
                        if diag:
                            c0 = 512 * ki + 128 * (kb - 4 * j)
                            for hl in range(2):
                                nc.tensor.matmul(
                                    pss[hl][:, c0 : c0 + 128],
                                    identf[:],
                                    utri[:],
                                    start=True,
                                    stop=False,
                                )
                        for hl in range(2):
                            h0 = 64 * hl
                            nc.tensor.matmul(
                                pss[hl][:, 512 * ki : 512 * (ki + 1)],
                                kd[h0 : h0 + 64, 128 * kb : 128 * (kb + 1)],
                                qd[h0 : h0 + 64, q0 : q0 + 512],
                                start=not diag,
                                stop=True,
                            )

                    def queue_group(j, grp, dst, qd=qd, kd=kd):
                        """Queue closures emitting one S group + its exps."""
                        st = {}

                        def cl_a():
                            pss = {}
                            for hl in range(2):
                                pss[hl] = psS.tile(
                                    [128, 1024], F32, tag="s", name=f"s{hl}"
                                )
                            st["pss"] = pss
                            s_ki(j, grp, 0, pss, qd, kd)

                        def cl_b():
                            s_ki(j, grp, 1, st["pss"], qd, kd)

                        def cl_c():
                            for hl in range(2):
                                pt = ppool.tile([128, 1024], F16, tag="p", name="pt")
                                nc.scalar.activation(
                                    pt[:],
                                    st["pss"][hl][:],
                                    mybir.ActivationFunctionType.Exp,
                                    scale=EXP_SCALE,
                                )
                                dst[(hl, grp)] = pt

                        filler.extend([cl_a, cl_b, cl_c])

                    # queue this fg's S/exp work; actual emission is pumped
                    # in fine grains between PV matmuls so the PE stream
                    # stays dense (keeps the HAM clock warm through the
                    # LDWEIGHTS-heavy PV stretches)
                    p_by_j = [dict() for _ in range(NQC)]
                    for j in range(NQC):
                        for grp in range(2 * (j + 1)):
                            queue_group(j, grp, p_by_j[j])
                    # next fg's projection chains go behind this fg's S work
                    # (they drain during the long final PV stretch)
                    if fg + 1 < NFG:
                        for pname in ("q", "k"):
                            for tch in range(4):
                                filler.append(
                                    lambda p=pname, f=fg + 1, t=tch: qk_chain(
                                        p, f, t, qkd[(f, p)],
                                        (bqs if p == "q" else bks)[f])
                                )

                    def ensure(dst, upto_grp):
                        while any(
                            (hl, g) not in dst
                            for g in range(upto_grp + 1)
                            for hl in range(2)
                        ):
                            assert filler, "dense work queue underflow"
                            fill(1)

                    for j in range(NQC):
                        p_tiles = p_by_j[j]
                        for i in range(4):
                            ensure(p_tiles, 2 * j + i // 2)
                            qt = 4 * j + i
                            pso = psO.tile([128, 130], F32, tag="o")
                            nkb = 4 * j + i
                            for hl in range(2):
                                for kb in range(nkb + 1):
                                    grp, ki = kb // 2, kb % 2
                                    c0 = 512 * ki + 128 * i
                                    nc.tensor.matmul(
                                        pso[:, 65 * hl : 65 * hl + 65],
                                        p_tiles[(hl, grp)][:, c0 : c0 + 128],
                                        vstore[kb][:, 130 * fg + 65 * hl : 130 * fg + 65 * hl + 65],
                                        start=(kb == 0),
                                        stop=(kb == nkb),
                                    )
                                    if kb % 4 == 3:
                                        fill(1)
                            psv = pso[:].rearrange("p (h c) -> p h c", h=2)
                            rec = mpool.tile([128, 2], F32, tag="rec")
                            nc.vector.reciprocal(rec[:], psv[:, :, 64])
                            ot = mpool.tile([128, 128], F16, tag="otok")
                            rec_b = bass.AP(
                                rec[:].tensor, rec[:].offset,
                                [rec[:].ap[0], [1, 2], [0, 64]],
                            )
                            nc.vector.tensor_tensor(
                                ot[:].rearrange("p (h c) -> p h c", h=2),
                                psv[:, :, 0:64],
                                rec_b,
                                mybir.AluOpType.mult,
                            )
                            pst = psO.tile([128, 128], F16, tag="o")
                            nc.tensor.transpose(pst[:], ot[:], identf[:])
                            nc.vector.tensor_copy(
                                o_feat[fg][:, 128 * qt : 128 * (qt + 1)], pst[:]
                            )
                            fill(1)

                    # own-half staging for the out projection
                    at = apool.tile([128, TT2], F16, tag=f"attown{fg}")
                    if sim_mode:
                        nc.sync.dma_start(at[:], o_feat[fg][:, 0:TT2])
                    else:
                        nc.gpsimd.dma_start(at[:], o_feat[fg][:, ds(poff, TT2)])
                    att_own.append(at)

                    # pairwise exchange of the partner-needed halves, split
                    # into three collectives (fg 0-1 mid-kernel, fg2, fg3)
                    # so only the last small one can expose latency
                    if fg in (1, 2, 3):
                        fgis = (0, 1) if fg == 1 else (fg,)
                        nr = 128 * len(fgis)
                        cc_in = dram.tile([nr, TT2], F16)
                        cc_out = dram.tile([2, nr, TT2], F16)
                        for k, fgi in enumerate(fgis):
                            if sim_mode:
                                nc.sync.dma_start(
                                    cc_in[128 * k : 128 * (k + 1), :],
                                    o_feat[fgi][:, TT2 : T],
                                )
                            else:
                                nc.gpsimd.dma_start(
                                    cc_in[128 * k : 128 * (k + 1), :],
                                    o_feat[fgi][:, ds(opp, TT2)],
                                )
                        if sim_mode:
                            nc.sync.dma_start(cc_out[0], cc_in[:])
                            nc.sync.dma_start(cc_out[1], cc_in[:])
                        else:
                            nc.gpsimd.collective_compute(
                                "AllGather",
                                mybir.AluOpType.bypass,
                                replica_groups=[[0, 1], [2, 3], [4, 5], [6, 7]],
                                ins=[cc_in.opt()],
                                outs=[cc_out.opt()],
                            )
                        cc_bufs.append((cc_out, nr))

                # drain any leftover filler work
                fill(len(filler))
                _ps_stack.close()  # free attention psum banks for out-proj

                # partner-half attention tiles from the exchanged buffers
                att_p = []
                for bi, (cc_out, nr) in enumerate(cc_bufs):
                    cc_flat = cc_out[:].rearrange("s p t -> (s p) t")
                    pbase = nr if sim_mode else ((pid + 1) % 2) * nr
                    for k in range(nr // 128):
                        at = apool.tile([128, TT2], F16, tag=f"attp{len(att_p)}")
                        if sim_mode:
                            nc.gpsimd.dma_start(
                                at[:], cc_flat[nr + 128 * k : nr + 128 * (k + 1), :]
                            )
                        else:
                            nc.gpsimd.dma_start(
                                at[:], cc_flat[ds(pbase + 128 * k, 128), :]
                            )
                        att_p.append(at)

                # out projection: own-half products first, partner after.
                # Wide psum ring so several chains can accumulate their
                # own-half products while the last exchange is in flight.
                psB = _ps_stack.enter_context(
                    tc.tile_pool(name="psB", bufs=5, space="PSUM")
                )
                for dt_ in range(8):
                    for tch in range(2):
                        t0 = 512 * tch
                        ps = psB.tile([128, 512], F32, tag="oproj")
                        for fs in range(4):
                            nc.tensor.matmul(
                                ps[:],
                                woa[fs][:, 128 * dt_ : 128 * (dt_ + 1)],
                                att_own[fs][:, t0 : t0 + 512],
                                start=(fs == 0),
                                stop=False,
                            )
                        for fs in range(4):
                            nc.tensor.matmul(
                                ps[:],
                                wob[fs][:, 128 * dt_ : 128 * (dt_ + 1)],
                                att_p[fs][:, t0 : t0 + 512],
                                start=False,
                                stop=(fs == 3),
                            )
                        ob = outpool.tile([128, 512], F32, tag="ob")
                        nc.vector.tensor_scalar_add(ob[:], ps[:], bos[dt_][:])
                        nc.sync.dma_start(
                            out_d[128 * dt_ : 128 * (dt_ + 1), t0 : t0 + 512], ob[:]
                        )
                _ps_stack.close()

    nc.compile()
    return nc


def _prep_inputs(x, Wq, bq, Wk, bk, Wv, bv, Wo, bo):
    """Build the 8 per-core input maps."""
    x = np.asarray(x)
    ident = np.eye(128, dtype=np.float32)
    r = np.arange(128)
    utri = np.where(r[None, :] < r[:, None], np.float32(MASK_NEG), np.float32(0.0))
    bo_r = np.asarray(bo).astype(np.float32).reshape(8, 128, 1)
    Wo = np.asarray(Wo).astype(np.float16)

    in_maps = []
    for c in range(N_CORES):
        b = c // 2
        hs = (c % 2) * FH
        ps = FH - hs  # partner's feature offset
        in_maps.append(
            {
                "xT": np.ascontiguousarray(x[b].T).astype(np.float16),
                "wq": np.asarray(Wq)[:, hs : hs + FH].astype(np.float16),
                "wk": np.asarray(Wk)[:, hs : hs + FH].astype(np.float16),
                "wv": np.asarray(Wv)[:, hs : hs + FH].astype(np.float16),
                "woa": np.ascontiguousarray(Wo[hs : hs + FH, :]),
                "wob": np.ascontiguousarray(Wo[ps : ps + FH, :]),
                "bq": np.asarray(bq)[hs : hs + FH].astype(np.float32).reshape(4, 128, 1),
                "bk": np.asarray(bk)[hs : hs + FH].astype(np.float32).reshape(4, 128, 1),
                "bvr": np.asarray(bv)[hs : hs + FH].astype(np.float16).reshape(1, FH),
                "bo": bo_r,
                "identf": ident.astype(np.float16),
                "utri": utri.astype(np.float16),
            }
        )
    return in_maps


_NC_CACHE = None


def kernel(x, Wq, bq, Wk, bk, Wv, bv, Wo, bo):
    global _NC_CACHE
    if _NC_CACHE is None:
        _NC_CACHE = build_nc()
    nc = _NC_CACHE
    in_maps = _prep_inputs(x, Wq, bq, Wk, bk, Wv, bv, Wo, bo)
    res = bass_utils.run_bass_kernel_spmd(nc, in_maps, core_ids=list(range(N_CORES)))
    out = np.empty((B, T, D), dtype=np.float32)
    for c in range(N_CORES):
        b = c // 2
        half = c % 2
        out[b, half * TT2 : (half + 1) * TT2, :] = res.results[c]["out_T"].T
    return out
